# revision 8
# baseline (speedup 1.0000x reference)
"""Trainium2 Bass kernel for nn_AdaptiveSampler (sparse grid_sample attention).

Strategy (data-parallel over batch, 8 cores x 4 batch items each):
  - Host: features reshaped channels-last [B*H*W, C] in bf16 so every
    spatial cell is one contiguous 2KB row -> indirect row gathers.
  - Device per core:
      phase A: keypoint -> bilinear corner cells/weights (DVE f32 math)
      seed    = dma_gather(4 corners x 512 keypoints) -> weighted reduce
      MLPs    = PE matmuls (offsets + attention logits), softmax on DVE/ACT
      phase B: 16 corner cells/weights per keypoint (4 pts x 4 corners)
      fused   = per-batch dma_gather(2048 rows) * broadcast weights,
                segment-reduce over 16, PE-transpose to [j, c], DMA out.
All computation (gathers, MLPs, softmax, bilinear) happens on-device; the
host only reorders input layout and concatenates per-core outputs.
"""

import os
import sys
from contextlib import ExitStack

import numpy as np

sys.path.insert(0, "/opt/trn_rl_repo")

import ml_dtypes

import concourse.bass as bass
import concourse.tile as tile
from concourse import bacc, mybir

F32 = mybir.dt.float32
BF16 = mybir.dt.bfloat16
I16 = mybir.dt.int16

ALU = mybir.AluOpType
ACT = mybir.ActivationFunctionType
AX = mybir.AxisListType

B = 4          # batch items per core
C = 1024       # channels
H = W = 64
HW = H * W     # 4096 cells per batch item
J = 128        # keypoints
NP = 4         # sample points per keypoint
Q = C // 128   # 8 channel chunks
NIDX = J * 16  # 2048 indices per gather (seed: J*B*4 ; main: per-b J*16)
TWO23 = float(2 ** 23)


def _floor(nc, pool, src, shape):
    """floor(src) on DVE via round-to-nearest + correction. Returns tile."""
    rnd = pool.tile(list(shape), F32, tag="floor_rnd")
    nc.vector.tensor_scalar(rnd[:], src, TWO23, TWO23, ALU.add, ALU.subtract)
    flo = pool.tile(list(shape), F32, tag="floor_out")
    # flo = (src < rnd) ? 1 : 0 ; then flo = rnd - flo
    nc.vector.tensor_tensor(flo[:], src, rnd[:], ALU.is_lt)
    nc.vector.tensor_tensor(flo[:], rnd[:], flo[:], ALU.subtract)
    return flo


def build_nc():
    nc = bacc.Bacc()

    feat = nc.declare_dram_parameter("feat", [B * HW, C], BF16, isOutput=False)
    kp = nc.declare_dram_parameter("kp", [J, 2 * B], F32, isOutput=False)
    w1o = nc.declare_dram_parameter("w1o", [128, Q, 128], BF16, isOutput=False)
    w1a = nc.declare_dram_parameter("w1a", [128, Q, 128], BF16, isOutput=False)
    w2o = nc.declare_dram_parameter("w2o", [128, 8], BF16, isOutput=False)
    w2a = nc.declare_dram_parameter("w2a", [128, 4], BF16, isOutput=False)
    b1o = nc.declare_dram_parameter("b1o", [128, 1], F32, isOutput=False)
    b1a = nc.declare_dram_parameter("b1a", [128, 1], F32, isOutput=False)
    b2o = nc.declare_dram_parameter("b2o", [8, 1], F32, isOutput=False)
    b2a = nc.declare_dram_parameter("b2a", [4, 1], F32, isOutput=False)
    bbase = nc.declare_dram_parameter("bbase", [128, B], F32, isOutput=False)
    ident = nc.declare_dram_parameter("ident", [128, 128], F32, isOutput=False)
    out = nc.declare_dram_parameter("out", [B * J, C], F32, isOutput=True)

    # DRAM scratch for flattening per-column weights before partition bcast
    wscr = nc.dram_tensor("wscr", [B + 1, J * 16], BF16)

    with ExitStack() as ctx:
        tc = ctx.enter_context(tile.TileContext(nc))
        cons = ctx.enter_context(tc.tile_pool(name="cons", bufs=1))
        a = ctx.enter_context(tc.tile_pool(name="phaseA", bufs=1))
        gp = ctx.enter_context(tc.tile_pool(name="gather", bufs=2))
        wp = ctx.enter_context(tc.tile_pool(name="wbc", bufs=2))
        op = ctx.enter_context(tc.tile_pool(name="outT", bufs=2))
        ip = ctx.enter_context(tc.tile_pool(name="idxw", bufs=2))
        ps = ctx.enter_context(tc.tile_pool(name="psT", bufs=3, space="PSUM"))
        pmm = ctx.enter_context(tc.tile_pool(name="psMM", bufs=3, space="PSUM"))

        # ---------------- constants ----------------
        kp_sb = cons.tile([J, B, 2], F32)
        nc.sync.dma_start(out=kp_sb[:], in_=kp[:].rearrange("j (b t) -> j b t", t=2))
        w1o_sb = cons.tile([128, Q, 128], BF16)
        nc.sync.dma_start(out=w1o_sb[:], in_=w1o[:])
        w1a_sb = cons.tile([128, Q, 128], BF16)
        nc.sync.dma_start(out=w1a_sb[:], in_=w1a[:])
        w2o_sb = cons.tile([128, 8], BF16)
        nc.sync.dma_start(out=w2o_sb[:], in_=w2o[:])
        w2a_sb = cons.tile([128, 4], BF16)
        nc.sync.dma_start(out=w2a_sb[:], in_=w2a[:])
        b1o_sb = cons.tile([128, 1], F32)
        nc.sync.dma_start(out=b1o_sb[:], in_=b1o[:])
        b1a_sb = cons.tile([128, 1], F32)
        nc.sync.dma_start(out=b1a_sb[:], in_=b1a[:])
        b2o_sb = cons.tile([8, 1], F32)
        nc.sync.dma_start(out=b2o_sb[:], in_=b2o[:])
        b2a_sb = cons.tile([4, 1], F32)
        nc.sync.dma_start(out=b2a_sb[:], in_=b2a[:])
        bbase_sb = cons.tile([128, B], F32)
        nc.sync.dma_start(out=bbase_sb[:], in_=bbase[:])
        id_sb = cons.tile([128, 128], F32)
        nc.sync.dma_start(out=id_sb[:], in_=ident[:])

        # ---------------- phase A: seed corners ----------------
        # pixel coords: ix = (x+1)*31.5, [J, B]
        ix = a.tile([J, B], F32)
        nc.vector.tensor_scalar(ix[:], kp_sb[:, :, 0], 31.5, 31.5, ALU.mult, ALU.add)
        iy = a.tile([J, B], F32)
        nc.vector.tensor_scalar(iy[:], kp_sb[:, :, 1], 31.5, 31.5, ALU.mult, ALU.add)

        x0 = _floor(nc, a, ix[:], (J, B))
        y0 = _floor(nc, a, iy[:], (J, B))

        # xp [J,B,2] = (x0, x0+1); wxp [J,B,2] = (1-fx, fx); same for y
        def pair_and_weights(base, i_coord, tagp):
            p = a.tile([J, B, 2], F32, tag=f"{tagp}_p")
            wgt = a.tile([J, B, 2], F32, tag=f"{tagp}_w")
            nc.vector.tensor_copy(p[:, :, 0], base[:])
            nc.vector.tensor_scalar_add(p[:, :, 1], base[:], 1.0)
            # fx = i - x0 -> wgt1 ; wgt0 = 1 - fx
            nc.vector.tensor_tensor(wgt[:, :, 1], i_coord, base[:], ALU.subtract)
            nc.vector.tensor_scalar(
                wgt[:, :, 0], wgt[:, :, 1], -1.0, 1.0, ALU.mult, ALU.add
            )
            return p, wgt

        xp, wxp = pair_and_weights(x0, ix[:], "x")
        yp, wyp = pair_and_weights(y0, iy[:], "y")

        # seed cell idx [J, B, 2cy, 2cx] = bbase + yp*64 + xp
        idx4 = a.tile([J, B, 2, 2], F32)
        t1 = a.tile([J, B, 2], F32)
        nc.vector.tensor_scalar_mul(t1[:], yp[:], 64.0)
        nc.vector.tensor_tensor(
            idx4[:],
            t1[:].unsqueeze(3).to_broadcast((J, B, 2, 2)),
            xp[:].unsqueeze(2).to_broadcast((J, B, 2, 2)),
            ALU.add,
        )
        nc.vector.tensor_tensor(
            idx4[:],
            idx4[:],
            bbase_sb[:].unsqueeze(2).unsqueeze(3).to_broadcast((J, B, 2, 2)),
            ALU.add,
        )
        w4 = a.tile([J, B, 2, 2], F32)
        nc.vector.tensor_tensor(
            w4[:],
            wyp[:].unsqueeze(3).to_broadcast((J, B, 2, 2)),
            wxp[:].unsqueeze(2).to_broadcast((J, B, 2, 2)),
            ALU.mult,
        )

        def wrap_idx(idx_flat_ap):
            """[J,16] f32 cell ids -> wrapped+replicated [128, J] int16 tile."""
            rep = ip.tile([J, 8, 16], F32, tag="idxrep")
            for g in range(8):
                nc.vector.tensor_copy(rep[:, g, :], idx_flat_ap)
            psT = ps.tile([128, J], F32, tag="tp")
            nc.tensor.transpose(
                psT[:], rep[:].rearrange("j g c -> j (g c)"), id_sb[:, :J]
            )
            idxw = ip.tile([128, J], I16, tag="idxw")
            nc.vector.tensor_copy(idxw[:], psT[:])
            return idxw

        def bcast_weights(w_flat_ap, slot):
            """[J,16] f32 col-weights -> [128, NIDX] bf16 via DRAM bounce."""
            wb16 = a.tile([J, 16], BF16, tag=f"wb16_{slot}")
            nc.vector.tensor_copy(wb16[:], w_flat_ap)
            nc.sync.dma_start(
                out=wscr[slot].rearrange("(j c) -> j c", c=16), in_=wb16[:]
            )
            wbc = wp.tile([128, NIDX], BF16, tag="wbc")
            nc.sync.dma_start(
                out=wbc[:],
                in_=wscr[slot].unsqueeze(0).to_broadcast((128, NIDX)),
            )
            return wbc

        idxw_seed = wrap_idx(idx4[:].rearrange("j b cy cx -> j (b cy cx)"))
        wbc_seed = bcast_weights(w4[:].rearrange("j b cy cx -> j (b cy cx)"), B)

        # ---------------- seed gather + combine ----------------
        HN = NIDX // 4   # 512 indices per gather chunk (HW desc-ring limit)
        seed = a.tile([128, Q, J * B], BF16)
        for h in range(4):
            seedg = gp.tile([128, Q, HN], BF16, tag="seedg")
            nc.gpsimd.dma_gather(
                seedg[:],
                feat[:],
                idxw_seed[:, 32 * h : 32 * h + 32],
                num_idxs=HN,
                num_idxs_reg=HN,
                elem_size=C,
                transpose=True,
            )
            with nc.allow_low_precision("bf16 grid-sample compute"):
                nc.vector.tensor_tensor(
                    seedg[:],
                    seedg[:],
                    wbc_seed[:, HN * h : HN * (h + 1)]
                    .unsqueeze(1)
                    .to_broadcast((128, Q, HN)),
                    ALU.mult,
                )
                nc.vector.tensor_reduce(
                    seed[:, :, 128 * h : 128 * (h + 1)],
                    seedg[:].rearrange("p q (jb c) -> p q jb c", c=4),
                    AX.X,
                    ALU.add,
                )

        # ---------------- MLPs ----------------
        def mlp_head(w1_sb, b1_sb, name):
            hps = pmm.tile([128, J * B], F32, tag="mm")
            for q in range(Q):
                nc.tensor.matmul(
                    hps[:],
                    w1_sb[:, q, :],
                    seed[:, q, :],
                    start=(q == 0),
                    stop=(q == Q - 1),
                )
            h_sb = a.tile([128, J * B], BF16, tag=f"hsb_{name}")
            nc.scalar.activation(h_sb[:], hps[:], ACT.Relu, bias=b1_sb[:])
            return h_sb

        h_off = mlp_head(w1o_sb, b1o_sb, "off")
        h_att = mlp_head(w1a_sb, b1a_sb, "att")

        ops2 = pmm.tile([8, J * B], F32, tag="mm")
        nc.tensor.matmul(ops2[:], w2o_sb[:], h_off[:], start=True, stop=True)
        off2 = a.tile([8, J * B], F32)
        nc.scalar.activation(off2[:], ops2[:], ACT.Identity, bias=b2o_sb[:])

        aps2 = pmm.tile([4, J * B], F32, tag="mm")
        nc.tensor.matmul(aps2[:], w2a_sb[:], h_att[:], start=True, stop=True)
        att2 = a.tile([4, J * B], F32)
        nc.scalar.activation(att2[:], aps2[:], ACT.Identity, bias=b2a_sb[:])

        # transpose MLP outputs back to [J, B, ch] layout (per-b strided cols)
        offT = a.tile([J, B, 8], F32)
        attT = a.tile([J, B, 4], F32)
        for b in range(B):
            pso = ps.tile([J, 8], F32, tag="tp")
            nc.tensor.transpose(pso[:], off2[:, b::B], id_sb[:8, :8])
            nc.scalar.copy(offT[:, b, :], pso[:])
            psa = ps.tile([J, 4], F32, tag="tp")
            nc.tensor.transpose(psa[:], att2[:, b::B], id_sb[:4, :4])
            nc.scalar.copy(attT[:, b, :], psa[:])

        # ---------------- phase B: per-point corners ----------------
        # px/py [J, B, NP]
        px = a.tile([J, B, NP], F32)
        nc.vector.tensor_tensor(
            px[:],
            ix[:].unsqueeze(2).to_broadcast((J, B, NP)),
            offT[:, :, 0:NP],
            ALU.add,
        )
        py = a.tile([J, B, NP], F32)
        nc.vector.tensor_tensor(
            py[:],
            iy[:].unsqueeze(2).to_broadcast((J, B, NP)),
            offT[:, :, NP : 2 * NP],
            ALU.add,
        )

        def corner2(pc, tagp):
            """coords pc [J,B,NP] -> clamped pair [J,B,NP,2], masked wpair."""
            c0 = _floor(nc, a, pc[:], (J, B, NP))
            pair = a.tile([J, B, NP, 2], F32, tag=f"{tagp}_pair")
            wgt = a.tile([J, B, NP, 2], F32, tag=f"{tagp}_wgt")
            vmask = a.tile([J, B, NP], F32, tag=f"{tagp}_v")
            # frac -> w1; w0 = 1-frac
            nc.vector.tensor_tensor(wgt[:, :, :, 1], pc[:], c0[:], ALU.subtract)
            nc.vector.tensor_scalar(
                wgt[:, :, :, 0], wgt[:, :, :, 1], -1.0, 1.0, ALU.mult, ALU.add
            )
            # validity: c0 >= 0 ; c1 = c0+1 <= 63
            nc.vector.tensor_scalar(vmask[:], c0[:], 0.0, None, ALU.is_ge)
            nc.vector.tensor_tensor(wgt[:, :, :, 0], wgt[:, :, :, 0], vmask[:], ALU.mult)
            nc.vector.tensor_scalar(vmask[:], c0[:], 62.5, None, ALU.is_lt)
            nc.vector.tensor_tensor(wgt[:, :, :, 1], wgt[:, :, :, 1], vmask[:], ALU.mult)
            # clamped coords
            nc.vector.tensor_scalar_max(pair[:, :, :, 0], c0[:], 0.0)
            nc.vector.tensor_scalar(
                pair[:, :, :, 1], c0[:], 1.0, 63.0, ALU.add, ALU.min
            )
            return pair, wgt

        xpair, wxg = corner2(px, "px")
        ypair, wyg = corner2(py, "py")

        # softmax over NP  [J, B, NP]
        amax = a.tile([J, B, 1], F32)
        nc.vector.tensor_reduce(amax[:], attT[:], AX.X, ALU.max)
        ae = a.tile([J, B, NP], F32)
        nc.vector.tensor_tensor(
            ae[:], attT[:], amax[:].to_broadcast((J, B, NP)), ALU.subtract
        )
        nc.scalar.activation(ae[:], ae[:], ACT.Exp)
        asum = a.tile([J, B, 1], F32)
        nc.vector.tensor_reduce(asum[:], ae[:], AX.X, ALU.add)
        nc.vector.reciprocal(asum[:], asum[:])
        attw = a.tile([J, B, NP], F32)
        nc.vector.tensor_tensor(
            attw[:], ae[:], asum[:].to_broadcast((J, B, NP)), ALU.mult
        )

        # combined weights [J, B, NP, 2cy, 2cx] and cells
        s1 = a.tile([J, B, NP, 2], F32)
        nc.vector.tensor_tensor(
            s1[:], attw[:].unsqueeze(3).to_broadcast((J, B, NP, 2)), wyg[:], ALU.mult
        )
        BN = B * NP
        w16 = a.tile([J, B, NP, 2, 2], F32)
        nc.vector.tensor_tensor(
            w16[:].rearrange("j b n cy cx -> j (b n) cy cx"),
            s1[:].rearrange("j b n cy -> j (b n) cy")
            .unsqueeze(3)
            .to_broadcast((J, BN, 2, 2)),
            wxg[:].rearrange("j b n cx -> j (b n) cx")
            .unsqueeze(2)
            .to_broadcast((J, BN, 2, 2)),
            ALU.mult,
        )
        idx16 = a.tile([J, B, NP, 2, 2], F32)
        t2 = a.tile([J, B, NP, 2], F32)
        nc.vector.tensor_scalar_mul(t2[:], ypair[:], 64.0)
        nc.vector.tensor_tensor(
            idx16[:].rearrange("j b n cy cx -> j (b n) cy cx"),
            t2[:].rearrange("j b n cy -> j (b n) cy")
            .unsqueeze(3)
            .to_broadcast((J, BN, 2, 2)),
            xpair[:].rearrange("j b n cx -> j (b n) cx")
            .unsqueeze(2)
            .to_broadcast((J, BN, 2, 2)),
            ALU.add,
        )
        nc.vector.tensor_tensor(
            idx16[:].rearrange("j b n cy cx -> j b (n cy cx)"),
            idx16[:].rearrange("j b n cy cx -> j b (n cy cx)"),
            bbase_sb[:].unsqueeze(2).to_broadcast((J, B, 4 * NP)),
            ALU.add,
        )

        # ---------------- phase B: gather + fuse per batch item ----------------
        for b in range(B):
            idxw_b = wrap_idx(
                idx16[:, b, :, :, :].rearrange("j n cy cx -> j (n cy cx)")
            )
            wbc_b = bcast_weights(
                w16[:, b, :, :, :].rearrange("j n cy cx -> j (n cy cx)"), b
            )
            fused = gp.tile([128, Q, J], F32, tag="fused")
            for h in range(4):
                mg = gp.tile([128, Q, HN], BF16, tag="mg")
                nc.gpsimd.dma_gather(
                    mg[:],
                    feat[:],
                    idxw_b[:, 32 * h : 32 * h + 32],
                    num_idxs=HN,
                    num_idxs_reg=HN,
                    elem_size=C,
                    transpose=True,
                )
                with nc.allow_low_precision("bf16 grid-sample compute"):
                    nc.vector.tensor_tensor(
                        mg[:],
                        mg[:],
                        wbc_b[:, HN * h : HN * (h + 1)]
                        .unsqueeze(1)
                        .to_broadcast((128, Q, HN)),
                        ALU.mult,
                    )
                nc.vector.tensor_reduce(
                    fused[:, :, 32 * h : 32 * (h + 1)],
                    mg[:].rearrange("p q (j c) -> p q j c", c=16),
                    AX.X,
                    ALU.add,
                )
            outT = op.tile([J, Q, 128], F32, tag="outT")
            for q in range(Q):
                pst = ps.tile([J, 128], F32, tag="tp")
                nc.tensor.transpose(pst[:], fused[:, q, :], id_sb[:, :J])
                nc.scalar.copy(outT[:, q, :], pst[:])
            nc.sync.dma_start(
                out=out[b * J : (b + 1) * J, :].rearrange(
                    "j (q c) -> j q c", q=Q
                ),
                in_=outT[:],
            )

    nc.finalize()
    return nc


def prepare_in_maps(features, keypoint_coords, w_off1, b_off1, w_off2, b_off2,
                    w_att1, b_att1, w_att2, b_att2, n_cores=8):
    bf = ml_dtypes.bfloat16
    f32 = np.float32

    def w1t(w):  # [128, C] -> [128 k_local, Q, 128 m] bf16
        return np.ascontiguousarray(
            w.T.reshape(Q, 128, 128).transpose(1, 0, 2).astype(bf)
        )

    w1o_h = w1t(np.asarray(w_off1, f32))
    w1a_h = w1t(np.asarray(w_att1, f32))
    w2o_h = np.ascontiguousarray(
        np.concatenate([w_off2[0::2], w_off2[1::2]], 0).T.astype(bf)
    )
    w2a_h = np.ascontiguousarray(np.asarray(w_att2, f32).T.astype(bf))
    b1o_h = np.asarray(b_off1, f32).reshape(128, 1).copy()
    b1a_h = np.asarray(b_att1, f32).reshape(128, 1).copy()
    b2o_h = np.concatenate([b_off2[0::2], b_off2[1::2]]).astype(f32).reshape(8, 1)
    b2a_h = np.asarray(b_att2, f32).reshape(4, 1).copy()
    bbase_h = np.broadcast_to(
        (np.arange(B, dtype=f32) * HW)[None, :], (128, B)
    ).copy()
    ident_h = np.eye(128, dtype=f32)

    in_maps = []
    for m in range(n_cores):
        bs = slice(B * m, B * (m + 1))
        feat_h = np.ascontiguousarray(
            np.asarray(features[bs], f32).transpose(0, 2, 3, 1).reshape(B * HW, C)
        ).astype(bf)
        kp_h = np.ascontiguousarray(
            np.asarray(keypoint_coords[bs], f32).transpose(1, 0, 2).reshape(J, 2 * B)
        )
        in_maps.append({
            "feat": feat_h, "kp": kp_h,
            "w1o": w1o_h, "w1a": w1a_h, "w2o": w2o_h, "w2a": w2a_h,
            "b1o": b1o_h, "b1a": b1a_h, "b2o": b2o_h, "b2a": b2a_h,
            "bbase": bbase_h, "ident": ident_h,
        })
    return in_maps


_NC_CACHE = None


def get_nc():
    global _NC_CACHE
    if _NC_CACHE is None:
        _NC_CACHE = build_nc()
    return _NC_CACHE


def kernel(**inputs):
    from concourse.bass_utils import run_bass_kernel_spmd

    n_cores = 8
    nc = get_nc()
    in_maps = prepare_in_maps(**inputs, n_cores=n_cores)
    res = run_bass_kernel_spmd(
        nc, in_maps, core_ids=list(range(n_cores)),
        trace=bool(int(os.environ.get("KERNEL_TRACE", "0") or 0)),
    )
    kernel.last_results = res
    outs = [r["out"].reshape(B, J, C) for r in res.results]
    return np.concatenate(outs, axis=0).astype(np.float32)
